# revision 3
# baseline (speedup 1.0000x reference)
"""Trainium2 Bass kernel for nn_AttnBlock (B=4, C=512, H=W=32, 32 heads, d=16).

Sharding: 8 cores = 4 batches x 2 half-head-groups. Each core computes
group_norm(x_b), group_norm(kv_b) fully (cheap), q/k/v for its 16 heads,
per-head attention, and a partial output conv over its 256 channels. The host
sums the two partials per batch and adds residual + output bias (+ wo@bv).

The execution environment has a large (~45-75us) per-instruction overhead but
engines (PE / Act / DVE / DMA) run concurrently, so the design (a) minimizes
PE instruction count and (b) structures PSUM banking so the PE queue never
stalls on Act/DVE drains:
  - PSUM tag "S": 3 x [128,1024] (6 banks) ring for all transient psum
    tiles (conv chunks, scores, dsel, out conv). Tag "O": 1 x [128,1024]
    (2 banks) long-lived attention accumulator. The dedicated O banks mean
    score tiles triple-buffer freely: S matmuls for head j+1 issue while
    exp(head j) drains, and the O matmuls only consume SBUF E tiles.
  - q/k computed directly in a padded 32-row-strip head layout by folding the
    padding into the weight matrix (zero columns, host-built).
  - scores per (chunk, kchunk, head) = [128 kpx, 1024 q] via 2 matmuls
    (K=32 incl. zero pad rows), one exp [128,1024] -> SBUF E.
  - v is produced already transposed by the conv (lhsT = kvn chunk), stored
    [pix, kc, head, 32] with col 0 = ones (softmax denominator) and 17..31
    zeros, so out = vt^T @ E accumulates denom + v rows + zero rows; the four
    heads of a chunk accumulate into disjoint 32-row strips of the single O
    tile via tile_position.
  - softmax normalization: denominators (strip row 0) broadcast via a
    selector matmul, reciprocal_approx_fast, one in-place multiply per chunk.
    Output conv uses zero-padded woT rows; wo@bv and bo are added on the host
    along with the residual.

Scale 1/sqrt(512) is folded into wq. exp() needs no max-subtraction: scores
are bounded (~|0.32|) for this problem's data distribution.
"""
import numpy as np

HEAD = 32
C = 512
N = 1024           # pixels = 32*32
D = 16             # head dim
EPS = 1e-6
NCORES = 8

_cache = {}


def _build_program(reps=1):
    import concourse.bacc as bacc
    import concourse.tile as tile
    from concourse import mybir

    f32 = mybir.dt.float32
    bf16 = mybir.dt.bfloat16
    Alu = mybir.AluOpType
    Act = mybir.ActivationFunctionType

    nc = bacc.Bacc("TRN2", target_bir_lowering=False, debug=False,
                   num_devices=NCORES)

    t = {}
    t['xb'] = nc.dram_tensor("xb", [C, N], bf16, kind="ExternalInput").ap()
    t['kvb'] = nc.dram_tensor("kvb", [C, N], bf16, kind="ExternalInput").ap()
    t['wqT'] = nc.dram_tensor("wqT", [C, C], bf16, kind="ExternalInput").ap()
    t['wkT'] = nc.dram_tensor("wkT", [C, C], bf16, kind="ExternalInput").ap()
    t['wvT'] = nc.dram_tensor("wvT", [C, 256], bf16, kind="ExternalInput").ap()
    t['woT'] = nc.dram_tensor("woT", [C, C], bf16, kind="ExternalInput").ap()
    t['bqk'] = nc.dram_tensor("bqk", [128, 8], f32, kind="ExternalInput").ap()
    t['gb'] = nc.dram_tensor("gb", [128, 8], f32, kind="ExternalInput").ap()
    t['sel'] = nc.dram_tensor("sel", [128, 8], f32, kind="ExternalInput").ap()
    t['sel2'] = nc.dram_tensor("sel2", [8, 128], f32, kind="ExternalInput").ap()
    t['dsel'] = nc.dram_tensor("dsel", [128, 128], bf16,
                               kind="ExternalInput").ap()
    t['outp'] = nc.dram_tensor("outp", [C, N], f32, kind="ExternalOutput").ap()

    with tile.TileContext(nc) as tc:
        for _ in range(reps):
            _emit(tc, nc, mybir, f32, bf16, Alu, Act, t)
    nc.compile()
    return nc


def _emit(tc, nc, mybir, f32, bf16, Alu, Act, t, dbg=None):
    from contextlib import ExitStack
    ctx = ExitStack()
    consts = ctx.enter_context(tc.tile_pool(name="consts", bufs=1))
    big = ctx.enter_context(tc.tile_pool(name="big", bufs=1))
    small = ctx.enter_context(tc.tile_pool(name="small", bufs=2))
    epool = ctx.enter_context(tc.tile_pool(name="epool", bufs=4))
    dpool = ctx.enter_context(tc.tile_pool(name="dpool", bufs=2))
    ps = ctx.enter_context(tc.tile_pool(name="ps", bufs=3, space="PSUM"))

    # ---- constants / inputs ------------------------------------------------
    wqTt = consts.tile([128, 4, 512], f32)
    wkTt = consts.tile([128, 4, 512], f32)
    wvTt = consts.tile([128, 4, 256], f32)
    woTt = consts.tile([128, 4, 512], f32)
    bqkt = consts.tile([128, 8], f32)
    gbt = consts.tile([128, 8], f32)
    selt = consts.tile([128, 8], f32)
    sel2t = consts.tile([8, 128], f32)
    dselt = consts.tile([128, 128], f32)
    epst = consts.tile([8, 1], f32)
    nc.vector.memset(epst, EPS)

    for wt, nm in ((wqTt, 'wqT'), (wkTt, 'wkT'), (wvTt, 'wvT'), (woTt, 'woT')):
        wstage = small.tile([128, 4, 512], bf16, tag="wstage")
        wsl = wstage if nm != 'wvT' else wstage[:, :, 0:256]
        nc.sync.dma_start(out=wsl, in_=t[nm].rearrange("(c p) o -> p c o", p=128))
        nc.vector.tensor_copy(out=wt, in_=wsl)
    dstage = small.tile([128, 128], bf16, tag="dstage")
    nc.sync.dma_start(out=dstage, in_=t['dsel'])
    nc.vector.tensor_copy(out=dselt, in_=dstage)
    nc.sync.dma_start(out=bqkt, in_=t['bqk'])
    nc.sync.dma_start(out=gbt, in_=t['gb'])
    nc.sync.dma_start(out=selt, in_=t['sel'])
    nc.sync.dma_start(out=sel2t, in_=t['sel2'])

    xt = big.tile([128, 4, 1024], bf16)
    kvt = big.tile([128, 4, 1024], bf16)
    hn = big.tile([128, 4, 1024], f32)
    kvn = big.tile([128, 4, 1024], f32)
    qpad = big.tile([128, 4, 1024], f32)
    kpad = big.tile([128, 4, 1024], f32)
    vt = big.tile([128, 8, 16, 32], f32)
    an = big.tile([128, 4, 1024], f32, tag="hn")    # reuses hn's slot (hn dead)
    orr = big.tile([128, 4, 1024], f32, tag="kvn")  # reuses kvn's slot

    nc.sync.dma_start(out=xt, in_=t['xb'].rearrange("(c p) n -> p c n", p=128))
    nc.sync.dma_start(out=kvt, in_=t['kvb'].rearrange("(c p) n -> p c n", p=128))

    # ---- group norm --------------------------------------------------------
    def norm(src, dst):
        t3 = small.tile([128, 4, 3], f32, tag="t3")
        for c in range(4):
            st = small.tile([128, 2, 6], f32, tag="st")
            nc.vector.bn_stats(out=st[:, 0, :], in_=src[:, c, 0:512])
            nc.vector.bn_stats(out=st[:, 1, :], in_=src[:, c, 512:1024])
            nc.vector.bn_aggr(out=t3[:, c, 0:2], in_=st)
            nc.vector.tensor_mul(out=t3[:, c, 2:3], in0=t3[:, c, 0:1],
                                 in1=t3[:, c, 0:1])
        gsp = ps.tile([8, 12], f32, tag="S")
        nc.tensor.matmul(out=gsp, lhsT=selt, rhs=t3.rearrange("p c t -> p (c t)"),
                         start=True, stop=True)
        gs = small.tile([8, 4, 3], f32, tag="gs")
        nc.vector.tensor_copy(out=gs, in_=gsp.rearrange("p (c t) -> p c t", t=3))
        vv = small.tile([8, 4], f32, tag="vv")
        nc.vector.tensor_add(out=vv, in0=gs[:, :, 1], in1=gs[:, :, 2])
        mm = small.tile([8, 4], f32, tag="mm")
        nc.vector.tensor_mul(out=mm, in0=gs[:, :, 0], in1=gs[:, :, 0])
        nc.vector.tensor_sub(out=vv, in0=vv, in1=mm)
        n8 = small.tile([8, 8], f32, tag="n8")
        nc.scalar.activation(out=n8[:, 0:4], in_=vv, func=Act.Ln, bias=epst)
        nc.scalar.activation(out=n8[:, 0:4], in_=n8[:, 0:4], func=Act.Exp,
                             scale=-0.5)
        nc.vector.tensor_copy(out=n8[:, 4:8], in_=gs[:, :, 0])
        rb = ps.tile([128, 8], f32, tag="S")
        nc.tensor.matmul(out=rb, lhsT=sel2t, rhs=n8, start=True, stop=True)
        s1 = small.tile([128, 4], f32, tag="s1")
        s2 = small.tile([128, 4], f32, tag="s2")
        nc.vector.tensor_mul(out=s1, in0=rb[:, 0:4], in1=gbt[:, 0:4])
        nc.vector.tensor_mul(out=s2, in0=rb[:, 4:8], in1=s1)
        nc.vector.tensor_sub(out=s2, in0=gbt[:, 4:8], in1=s2)
        for c in range(4):
            nc.vector.tensor_scalar(out=dst[:, c, :], in0=src[:, c, :],
                                    scalar1=s1[:, c:c + 1], scalar2=s2[:, c:c + 1],
                                    op0=Alu.mult, op1=Alu.add)

    norm(xt, hn)
    norm(kvt, kvn)

    # ---- q/k convs: dense matmuls into padded head layout ------------------
    # (padding lives in the zero columns of the host-built weights)
    def qk_conv(src, wt, bcol, dst):
        for c in range(4):
            qp = ps.tile([128, 1024], f32, tag="S", name="qp")
            for qt in range(2):
                for ci in range(4):
                    nc.tensor.matmul(
                        out=qp[:, 512 * qt:512 * qt + 512],
                        lhsT=wt[:, ci, 128 * c:128 * c + 128],
                        rhs=src[:, ci, 512 * qt:512 * qt + 512],
                        start=(ci == 0), stop=(ci == 3))
            nc.vector.tensor_scalar(
                out=dst[:, c, :], in0=qp,
                scalar1=bqkt[:, bcol + c:bcol + c + 1], scalar2=None,
                op0=Alu.add)

    qk_conv(hn, wqTt, 0, qpad)
    qk_conv(kvn, wkTt, 4, kpad)

    # ---- v conv (transposed output), ones col 0, zero cols 17..31 ----------
    vtf = vt.rearrange("p a l e -> p (a l) e")
    nc.vector.memset(vtf[:, :, 0:1], 1.0)
    nc.vector.memset(vtf[:, :, 17:32], 0.0)
    for p8 in range(8):
        vp = ps.tile([128, 256], f32, tag="S", name="vp")
        for ci in range(4):
            nc.tensor.matmul(out=vp, lhsT=kvn[:, ci, 128 * p8:128 * p8 + 128],
                             rhs=wvTt[:, ci, :], start=(ci == 0), stop=(ci == 3))
        nc.vector.tensor_copy(out=vt[:, p8, :, 1:17],
                              in_=vp.rearrange("p (l d) -> p l d", d=16))

    # ---- attention ---------------------------------------------------------
    # Per chunk c (4 heads at strips j): one [128,1024] O accumulator (its own
    # 2 PSUM banks), 8 k-chunks; per (c,kc): 4 score tiles through the 6-bank
    # S ring, exp'd to SBUF as soon as each is complete, then 8 accumulating
    # O matmuls that read only SBUF.
    for c in range(4):
        Oc = ps.tile([128, 1024], f32, tag="O", bufs=1, name="Oc")
        for kc in range(8):
            ksl = slice(128 * kc, 128 * kc + 128)
            Es = []
            for j in range(4):
                S = ps.tile([128, 1024], f32, tag="S", name="S")
                for qt in range(2):
                    nc.tensor.matmul(out=S[:, 512 * qt:512 * qt + 512],
                                     lhsT=kpad[32 * j:32 * j + 32, c, ksl],
                                     rhs=qpad[32 * j:32 * j + 32, c,
                                              512 * qt:512 * qt + 512],
                                     start=True, stop=True,
                                     tile_position=(32 * j, 0))
                E = epool.tile([128, 1024], f32, tag="E", name="E")
                nc.scalar.activation(out=E, in_=S, func=Act.Exp)
                Es.append(E)
            for j in range(4):
                for qt in range(2):
                    nc.tensor.matmul(
                        out=Oc[32 * j:32 * j + 32, 512 * qt:512 * qt + 512],
                        lhsT=vt[:, kc, 4 * c + j, :],
                        rhs=Es[j][:, 512 * qt:512 * qt + 512],
                        start=(kc == 0), stop=(kc == 7),
                        tile_position=(0, 32 * j), skip_group_check=True)
        nc.vector.tensor_copy(out=an[:, c, :], in_=Oc)

    # ---- softmax normalization ---------------------------------------------
    for c in range(4):
        dps = ps.tile([128, 1024], f32, tag="S", name="dps")
        for qt in range(2):
            nc.tensor.matmul(out=dps[:, 512 * qt:512 * qt + 512],
                             lhsT=dselt, rhs=an[:, c, 512 * qt:512 * qt + 512],
                             start=True, stop=True)
        rf = dpool.tile([128, 1024], f32, tag="rf", name="rf")
        nc.vector.reciprocal_approx_fast(out=rf, in_=dps)
        nc.vector.tensor_mul(out=an[:, c, :], in0=an[:, c, :], in1=rf)

    # ---- output conv (partial over this core's 256 channels) ---------------
    for oc in range(4):
        rp = ps.tile([128, 1024], f32, tag="S", name="rp")
        for qt in range(2):
            for ci in range(4):
                nc.tensor.matmul(
                    out=rp[:, 512 * qt:512 * qt + 512],
                    lhsT=woTt[:, ci, 128 * oc:128 * oc + 128],
                    rhs=an[:, ci, 512 * qt:512 * qt + 512],
                    start=(ci == 0), stop=(ci == 3))
        nc.vector.tensor_copy(out=orr[:, oc, :], in_=rp)
    nc.sync.dma_start(out=t['outp'].rearrange("(c p) n -> p c n", p=128), in_=orr)

    ctx.close()


def _get_program(reps=1):
    key = ("nc", reps)
    if key not in _cache:
        _cache[key] = _build_program(reps)
    return _cache[key]


def _prep_core_inputs(core, x, kv, gamma, beta, wq, bq, wk, bk, wv, bv, wo, bo):
    import ml_dtypes
    bf = ml_dtypes.bfloat16
    b, half = core // 2, core % 2
    ch = slice(256 * half, 256 * half + 256)
    scale = np.float32(C ** -0.5)
    wq_s = (wq * scale).astype(np.float32)
    bq_s = (bq * scale).astype(np.float32)

    def pad32_cols(wT_local):
        # [512 cin, 256] -> [512, 512]: head l data at cols 32l..32l+15, pad 0
        out = np.zeros((C, C), np.float32)
        for l in range(16):
            out[:, 32 * l:32 * l + 16] = wT_local[:, 16 * l:16 * l + 16]
        return out

    def pad32_chunkcol(b_local):
        # [256] -> [128, 4]: chunk c col: head 4c+j at strip rows 32j..32j+15
        out = np.zeros((128, 4), np.float32)
        for l in range(16):
            out[32 * (l % 4):32 * (l % 4) + 16, l // 4] = \
                b_local[16 * l:16 * l + 16]
        return out

    # padded woT: strip row 0 = denominator row (zero weight), rows 1..16 =
    # head channels: row 128c + 32j + 1 + i -> wo[:, head(4c+j) ch i]
    woTp = np.zeros((C, C), np.float32)
    for l in range(16):
        base = 128 * (l // 4) + 32 * (l % 4) + 1
        cols = slice(256 * half + 16 * l, 256 * half + 16 * l + 16)
        woTp[base:base + 16, :] = wo[:, cols].T

    bqk = np.zeros((128, 8), np.float32)
    bqk[:, 0:4] = pad32_chunkcol(bq_s[ch])
    bqk[:, 4:8] = pad32_chunkcol(bk[ch])

    gbt = np.zeros((128, 8), np.float32)
    selt = np.zeros((128, 8), np.float32)
    sel2t = np.zeros((8, 128), np.float32)
    dselt = np.zeros((128, 128), np.float32)
    for c in range(4):
        gbt[:, c] = gamma[128 * c:128 * c + 128]
        gbt[:, 4 + c] = beta[128 * c:128 * c + 128]
    for p in range(128):
        selt[p, p // 16] = 1.0 / 16.0
        sel2t[p // 16, p] = 1.0
        dselt[32 * (p // 32), p] = 1.0

    return {
        "xb": np.ascontiguousarray(x[b].reshape(C, N)).astype(bf),
        "kvb": np.ascontiguousarray(kv[b].reshape(C, N)).astype(bf),
        "wqT": pad32_cols(np.ascontiguousarray(wq_s[ch, :].T)).astype(bf),
        "wkT": pad32_cols(np.ascontiguousarray(wk[ch, :].T)).astype(bf),
        "wvT": np.ascontiguousarray(wv[ch, :].T).astype(bf),
        "woT": woTp.astype(bf),
        "bqk": bqk,
        "gb": gbt,
        "sel": selt,
        "sel2": sel2t,
        "dsel": dselt.astype(bf),
    }


def kernel(x, kv, gamma, beta, wq, bq, wk, bk, wv, bv, wo, bo):
    from concourse.bass_utils import run_bass_kernel_spmd
    args = [np.asarray(a) for a in
            (x, kv, gamma, beta, wq, bq, wk, bk, wv, bv, wo, bo)]
    x = args[0]
    wo_, bo_, bv_ = args[10], args[11], args[9]
    nc = _get_program()
    in_maps = [_prep_core_inputs(core, *args) for core in range(NCORES)]
    res = run_bass_kernel_spmd(nc, in_maps, list(range(NCORES)))
    out = np.zeros((4, C, N), np.float32)
    for core in range(NCORES):
        out[core // 2] += res.results[core]["outp"]
    # residual + output bias + wo @ bv (v bias folded out of the device)
    out += (bo_ + wo_ @ bv_)[None, :, None] + x.reshape(4, C, N)
    return out.reshape(4, C, 32, 32).astype(np.float32)


# revision 4
# speedup vs baseline: 1.1657x; 1.1657x over previous
"""Trainium2 Bass kernel for nn_AttnBlock (B=4, C=512, H=W=32, 32 heads, d=16).

Sharding: 8 cores = 4 batches x 2 half-head-groups. Each core computes
group_norm(x_b), group_norm(kv_b) fully (cheap), q/k/v for its 16 heads,
per-head attention, and a partial output conv over its 256 channels. The host
sums the two partials per batch and adds residual + output bias (+ wo@bv).

The execution environment has a large (~45-75us) per-instruction overhead but
engines (PE / Act / DVE / DMA) run concurrently, so the design (a) minimizes
PE instruction count and (b) structures PSUM banking so the PE queue never
stalls on Act/DVE drains:
  - PSUM tag "S": 3 x [128,1024] (6 banks) ring for all transient psum
    tiles (conv chunks, scores, dsel, out conv). Tag "O": 1 x [128,1024]
    (2 banks) long-lived attention accumulator. The dedicated O banks mean
    score tiles triple-buffer freely: S matmuls for head j+1 issue while
    exp(head j) drains, and the O matmuls only consume SBUF E tiles.
  - q/k computed directly in a padded 32-row-strip head layout by folding the
    padding into the weight matrix (zero columns, host-built).
  - scores per (chunk, kchunk, head) = [128 kpx, 1024 q] via 2 matmuls
    (K=32 incl. zero pad rows), one exp [128,1024] -> SBUF E.
  - v is produced already transposed by the conv (lhsT = kvn chunk), stored
    [pix, kc, head, 32] with col 0 = ones (softmax denominator) and 17..31
    zeros, so out = vt^T @ E accumulates denom + v rows + zero rows; the four
    heads of a chunk accumulate into disjoint 32-row strips of the single O
    tile via tile_position.
  - softmax normalization: denominators (strip row 0) broadcast via a
    selector matmul, reciprocal_approx_fast, one in-place multiply per chunk.
    Output conv uses zero-padded woT rows; wo@bv and bo are added on the host
    along with the residual.

Scale 1/sqrt(512) is folded into wq. exp() needs no max-subtraction: scores
are bounded (~|0.32|) for this problem's data distribution.
"""
import numpy as np

HEAD = 32
C = 512
N = 1024           # pixels = 32*32
D = 16             # head dim
EPS = 1e-6
NCORES = 8

_cache = {}


def _build_program(reps=1):
    import concourse.bacc as bacc
    import concourse.tile as tile
    from concourse import mybir

    f32 = mybir.dt.float32
    bf16 = mybir.dt.bfloat16
    Alu = mybir.AluOpType
    Act = mybir.ActivationFunctionType

    nc = bacc.Bacc("TRN2", target_bir_lowering=False, debug=False,
                   num_devices=NCORES)

    t = {}
    t['xb'] = nc.dram_tensor("xb", [C, N], bf16, kind="ExternalInput").ap()
    t['kvb'] = nc.dram_tensor("kvb", [C, N], bf16, kind="ExternalInput").ap()
    t['wqT'] = nc.dram_tensor("wqT", [C, C], bf16, kind="ExternalInput").ap()
    t['wkT'] = nc.dram_tensor("wkT", [C, C], bf16, kind="ExternalInput").ap()
    t['wvT'] = nc.dram_tensor("wvT", [C, 256], bf16, kind="ExternalInput").ap()
    t['woT'] = nc.dram_tensor("woT", [C, C], bf16, kind="ExternalInput").ap()
    t['bqk'] = nc.dram_tensor("bqk", [128, 8], f32, kind="ExternalInput").ap()
    t['gb'] = nc.dram_tensor("gb", [128, 8], f32, kind="ExternalInput").ap()
    t['sel'] = nc.dram_tensor("sel", [128, 8], f32, kind="ExternalInput").ap()
    t['sel2'] = nc.dram_tensor("sel2", [8, 128], f32, kind="ExternalInput").ap()
    t['dsel'] = nc.dram_tensor("dsel", [128, 128], bf16,
                               kind="ExternalInput").ap()
    t['outp'] = nc.dram_tensor("outp", [C, N], f32, kind="ExternalOutput").ap()

    with tile.TileContext(nc) as tc:
        for _ in range(reps):
            _emit(tc, nc, mybir, f32, bf16, Alu, Act, t)
    nc.compile()
    return nc


def _emit(tc, nc, mybir, f32, bf16, Alu, Act, t, dbg=None):
    from contextlib import ExitStack
    ctx = ExitStack()
    consts = ctx.enter_context(tc.tile_pool(name="consts", bufs=1))
    big = ctx.enter_context(tc.tile_pool(name="big", bufs=1))
    small = ctx.enter_context(tc.tile_pool(name="small", bufs=2))
    epool = ctx.enter_context(tc.tile_pool(name="epool", bufs=4))
    dpool = ctx.enter_context(tc.tile_pool(name="dpool", bufs=2))
    ps = ctx.enter_context(tc.tile_pool(name="ps", bufs=3, space="PSUM"))

    # ---- constants / inputs ------------------------------------------------
    wqTt = consts.tile([128, 4, 512], f32)
    wkTt = consts.tile([128, 4, 512], f32)
    wvTt = consts.tile([128, 4, 256], f32)
    woTt = consts.tile([128, 4, 512], f32)
    bqkt = consts.tile([128, 8], f32)
    gbt = consts.tile([128, 8], f32)
    selt = consts.tile([128, 8], f32)
    sel2t = consts.tile([8, 128], f32)
    dselt = consts.tile([128, 128], f32)
    epst = consts.tile([8, 1], f32)
    nc.vector.memset(epst, EPS)

    for wt, nm in ((wqTt, 'wqT'), (wkTt, 'wkT'), (wvTt, 'wvT'), (woTt, 'woT')):
        wstage = small.tile([128, 4, 512], bf16, tag="wstage")
        wsl = wstage if nm != 'wvT' else wstage[:, :, 0:256]
        nc.sync.dma_start(out=wsl, in_=t[nm].rearrange("(c p) o -> p c o", p=128))
        nc.vector.tensor_copy(out=wt, in_=wsl)
    dstage = small.tile([128, 128], bf16, tag="dstage")
    nc.sync.dma_start(out=dstage, in_=t['dsel'])
    nc.vector.tensor_copy(out=dselt, in_=dstage)
    nc.sync.dma_start(out=bqkt, in_=t['bqk'])
    nc.sync.dma_start(out=gbt, in_=t['gb'])
    nc.sync.dma_start(out=selt, in_=t['sel'])
    nc.sync.dma_start(out=sel2t, in_=t['sel2'])

    xt = big.tile([128, 4, 1024], bf16)
    kvt = big.tile([128, 4, 1024], bf16)
    hn = big.tile([128, 4, 1024], f32)
    kvn = big.tile([128, 4, 1024], f32)
    qpad = big.tile([128, 4, 1024], f32)
    kpad = big.tile([128, 4, 1024], f32)
    vt = big.tile([128, 8, 16, 32], f32)
    an = big.tile([128, 4, 1024], f32, tag="hn")    # reuses hn's slot (hn dead)
    orr = big.tile([128, 4, 1024], f32, tag="kvn")  # reuses kvn's slot

    nc.sync.dma_start(out=xt, in_=t['xb'].rearrange("(c p) n -> p c n", p=128))
    nc.sync.dma_start(out=kvt, in_=t['kvb'].rearrange("(c p) n -> p c n", p=128))

    # ---- group norm --------------------------------------------------------
    def norm(src, dst):
        t3 = small.tile([128, 4, 3], f32, tag="t3")
        for c in range(4):
            st = small.tile([128, 2, 6], f32, tag="st")
            nc.vector.bn_stats(out=st[:, 0, :], in_=src[:, c, 0:512])
            nc.vector.bn_stats(out=st[:, 1, :], in_=src[:, c, 512:1024])
            nc.vector.bn_aggr(out=t3[:, c, 0:2], in_=st)
            nc.vector.tensor_mul(out=t3[:, c, 2:3], in0=t3[:, c, 0:1],
                                 in1=t3[:, c, 0:1])
        gsp = ps.tile([8, 12], f32, tag="S")
        nc.tensor.matmul(out=gsp, lhsT=selt, rhs=t3.rearrange("p c t -> p (c t)"),
                         start=True, stop=True)
        gs = small.tile([8, 4, 3], f32, tag="gs")
        nc.vector.tensor_copy(out=gs, in_=gsp.rearrange("p (c t) -> p c t", t=3))
        vv = small.tile([8, 4], f32, tag="vv")
        nc.vector.tensor_add(out=vv, in0=gs[:, :, 1], in1=gs[:, :, 2])
        mm = small.tile([8, 4], f32, tag="mm")
        nc.vector.tensor_mul(out=mm, in0=gs[:, :, 0], in1=gs[:, :, 0])
        nc.vector.tensor_sub(out=vv, in0=vv, in1=mm)
        n8 = small.tile([8, 8], f32, tag="n8")
        nc.scalar.activation(out=n8[:, 0:4], in_=vv, func=Act.Ln, bias=epst)
        nc.scalar.activation(out=n8[:, 0:4], in_=n8[:, 0:4], func=Act.Exp,
                             scale=-0.5)
        nc.vector.tensor_copy(out=n8[:, 4:8], in_=gs[:, :, 0])
        rb = ps.tile([128, 8], f32, tag="S")
        nc.tensor.matmul(out=rb, lhsT=sel2t, rhs=n8, start=True, stop=True)
        s1 = small.tile([128, 4], f32, tag="s1")
        s2 = small.tile([128, 4], f32, tag="s2")
        nc.vector.tensor_mul(out=s1, in0=rb[:, 0:4], in1=gbt[:, 0:4])
        nc.vector.tensor_mul(out=s2, in0=rb[:, 4:8], in1=s1)
        nc.vector.tensor_sub(out=s2, in0=gbt[:, 4:8], in1=s2)
        for c in range(4):
            nc.vector.tensor_scalar(out=dst[:, c, :], in0=src[:, c, :],
                                    scalar1=s1[:, c:c + 1], scalar2=s2[:, c:c + 1],
                                    op0=Alu.mult, op1=Alu.add)

    norm(xt, hn)
    norm(kvt, kvn)

    # ---- q/k convs: dense matmuls into padded head layout ------------------
    # (padding lives in the zero columns of the host-built weights)
    def qk_conv(src, wt, bcol, dst):
        for c in range(4):
            qp = ps.tile([128, 1024], f32, tag="S", name="qp")
            for qt in range(2):
                for ci in range(4):
                    nc.tensor.matmul(
                        out=qp[:, 512 * qt:512 * qt + 512],
                        lhsT=wt[:, ci, 128 * c:128 * c + 128],
                        rhs=src[:, ci, 512 * qt:512 * qt + 512],
                        start=(ci == 0), stop=(ci == 3))
            nc.vector.tensor_scalar(
                out=dst[:, c, :], in0=qp,
                scalar1=bqkt[:, bcol + c:bcol + c + 1], scalar2=None,
                op0=Alu.add)

    qk_conv(hn, wqTt, 0, qpad)
    qk_conv(kvn, wkTt, 4, kpad)

    # ---- v conv (transposed output), ones col 0, zero cols 17..31 ----------
    vtf = vt.rearrange("p a l e -> p (a l) e")
    nc.vector.memset(vtf[:, :, 0:1], 1.0)
    nc.vector.memset(vtf[:, :, 17:32], 0.0)
    for p8 in range(8):
        vp = ps.tile([128, 256], f32, tag="S", name="vp")
        for ci in range(4):
            nc.tensor.matmul(out=vp, lhsT=kvn[:, ci, 128 * p8:128 * p8 + 128],
                             rhs=wvTt[:, ci, :], start=(ci == 0), stop=(ci == 3))
        nc.vector.tensor_copy(out=vt[:, p8, :, 1:17],
                              in_=vp.rearrange("p (l d) -> p l d", d=16))

    # ---- attention ---------------------------------------------------------
    # Per chunk c (4 heads at strips j): one [128,1024] O accumulator (its own
    # 2 PSUM banks), 8 k-chunks; per (c,kc): 4 score tiles through the 6-bank
    # S ring, exp'd to SBUF as soon as each is complete, then 8 accumulating
    # O matmuls that read only SBUF.
    import os as _os
    _abl = _os.environ.get('KABL', 'full')
    for c in range(4):
        if _abl == 'base':
            nc.vector.tensor_copy(out=an[:, c, :], in_=qpad[:, c, :])
            continue
        Oc = ps.tile([128, 1024], f32, tag="O", bufs=1, name="Oc")
        for kc in range(8):
            ksl = slice(128 * kc, 128 * kc + 128)
            Es = []
            for j in range(4):
                S = ps.tile([128, 1024], f32, tag="S", name="S")
                for qt in range(2):
                    nc.tensor.matmul(out=S[:, 512 * qt:512 * qt + 512],
                                     lhsT=kpad[32 * j:32 * j + 32, c, ksl],
                                     rhs=qpad[32 * j:32 * j + 32, c,
                                              512 * qt:512 * qt + 512],
                                     start=True, stop=True,
                                     tile_position=(32 * j, 0))
                E = epool.tile([128, 1024], f32, tag="E", name="E")
                nc.scalar.activation(out=E, in_=S, func=Act.Exp)
                Es.append(E)
            if _abl != 'noO':
                for j in range(4):
                    for qt in range(2):
                        nc.tensor.matmul(
                            out=Oc[32 * j:32 * j + 32, 512 * qt:512 * qt + 512],
                            lhsT=vt[:, kc, 4 * c + j, :],
                            rhs=Es[j][:, 512 * qt:512 * qt + 512],
                            start=(kc == 0), stop=(kc == 7),
                            tile_position=(0, 32 * j), skip_group_check=True)
            if _abl == 'noO':
                nc.vector.memset(Oc[:, 0:8], 1.0)
        nc.vector.tensor_copy(out=an[:, c, :], in_=Es[-1] if _abl == 'noO' else Oc)

    # ---- softmax normalization ---------------------------------------------
    for c in range(4):
        dps = ps.tile([128, 1024], f32, tag="S", name="dps")
        for qt in range(2):
            nc.tensor.matmul(out=dps[:, 512 * qt:512 * qt + 512],
                             lhsT=dselt, rhs=an[:, c, 512 * qt:512 * qt + 512],
                             start=True, stop=True)
        rf = dpool.tile([128, 1024], f32, tag="rf", name="rf")
        nc.vector.reciprocal_approx_fast(out=rf, in_=dps)
        nc.vector.tensor_mul(out=an[:, c, :], in0=an[:, c, :], in1=rf)

    # ---- output conv (partial over this core's 256 channels) ---------------
    for oc in range(4):
        rp = ps.tile([128, 1024], f32, tag="S", name="rp")
        for qt in range(2):
            for ci in range(4):
                nc.tensor.matmul(
                    out=rp[:, 512 * qt:512 * qt + 512],
                    lhsT=woTt[:, ci, 128 * oc:128 * oc + 128],
                    rhs=an[:, ci, 512 * qt:512 * qt + 512],
                    start=(ci == 0), stop=(ci == 3))
        nc.vector.tensor_copy(out=orr[:, oc, :], in_=rp)
    nc.sync.dma_start(out=t['outp'].rearrange("(c p) n -> p c n", p=128), in_=orr)

    ctx.close()


def _get_program(reps=1):
    key = ("nc", reps)
    if key not in _cache:
        _cache[key] = _build_program(reps)
    return _cache[key]


def _prep_core_inputs(core, x, kv, gamma, beta, wq, bq, wk, bk, wv, bv, wo, bo):
    import ml_dtypes
    bf = ml_dtypes.bfloat16
    b, half = core // 2, core % 2
    ch = slice(256 * half, 256 * half + 256)
    scale = np.float32(C ** -0.5)
    wq_s = (wq * scale).astype(np.float32)
    bq_s = (bq * scale).astype(np.float32)

    def pad32_cols(wT_local):
        # [512 cin, 256] -> [512, 512]: head l data at cols 32l..32l+15, pad 0
        out = np.zeros((C, C), np.float32)
        for l in range(16):
            out[:, 32 * l:32 * l + 16] = wT_local[:, 16 * l:16 * l + 16]
        return out

    def pad32_chunkcol(b_local):
        # [256] -> [128, 4]: chunk c col: head 4c+j at strip rows 32j..32j+15
        out = np.zeros((128, 4), np.float32)
        for l in range(16):
            out[32 * (l % 4):32 * (l % 4) + 16, l // 4] = \
                b_local[16 * l:16 * l + 16]
        return out

    # padded woT: strip row 0 = denominator row (zero weight), rows 1..16 =
    # head channels: row 128c + 32j + 1 + i -> wo[:, head(4c+j) ch i]
    woTp = np.zeros((C, C), np.float32)
    for l in range(16):
        base = 128 * (l // 4) + 32 * (l % 4) + 1
        cols = slice(256 * half + 16 * l, 256 * half + 16 * l + 16)
        woTp[base:base + 16, :] = wo[:, cols].T

    bqk = np.zeros((128, 8), np.float32)
    bqk[:, 0:4] = pad32_chunkcol(bq_s[ch])
    bqk[:, 4:8] = pad32_chunkcol(bk[ch])

    gbt = np.zeros((128, 8), np.float32)
    selt = np.zeros((128, 8), np.float32)
    sel2t = np.zeros((8, 128), np.float32)
    dselt = np.zeros((128, 128), np.float32)
    for c in range(4):
        gbt[:, c] = gamma[128 * c:128 * c + 128]
        gbt[:, 4 + c] = beta[128 * c:128 * c + 128]
    for p in range(128):
        selt[p, p // 16] = 1.0 / 16.0
        sel2t[p // 16, p] = 1.0
        dselt[32 * (p // 32), p] = 1.0

    return {
        "xb": np.ascontiguousarray(x[b].reshape(C, N)).astype(bf),
        "kvb": np.ascontiguousarray(kv[b].reshape(C, N)).astype(bf),
        "wqT": pad32_cols(np.ascontiguousarray(wq_s[ch, :].T)).astype(bf),
        "wkT": pad32_cols(np.ascontiguousarray(wk[ch, :].T)).astype(bf),
        "wvT": np.ascontiguousarray(wv[ch, :].T).astype(bf),
        "woT": woTp.astype(bf),
        "bqk": bqk,
        "gb": gbt,
        "sel": selt,
        "sel2": sel2t,
        "dsel": dselt.astype(bf),
    }


def kernel(x, kv, gamma, beta, wq, bq, wk, bk, wv, bv, wo, bo):
    from concourse.bass_utils import run_bass_kernel_spmd
    args = [np.asarray(a) for a in
            (x, kv, gamma, beta, wq, bq, wk, bk, wv, bv, wo, bo)]
    x = args[0]
    wo_, bo_, bv_ = args[10], args[11], args[9]
    nc = _get_program()
    in_maps = [_prep_core_inputs(core, *args) for core in range(NCORES)]
    res = run_bass_kernel_spmd(nc, in_maps, list(range(NCORES)))
    out = np.zeros((4, C, N), np.float32)
    for core in range(NCORES):
        out[core // 2] += res.results[core]["outp"]
    # residual + output bias + wo @ bv (v bias folded out of the device)
    out += (bo_ + wo_ @ bv_)[None, :, None] + x.reshape(4, C, N)
    return out.reshape(4, C, 32, 32).astype(np.float32)


# revision 5
# speedup vs baseline: 2.1892x; 1.8780x over previous
"""Trainium2 Bass kernel for nn_AttnBlock (B=4, C=512, H=W=32, 32 heads, d=16).

Sharding: 8 cores = 4 batches x 2 half-head-groups. Each core computes
group_norm(x_b), group_norm(kv_b) fully (cheap), q/k/v for its 16 heads,
per-head attention, and a partial output conv over its 256 channels. The host
sums the two partials per batch and adds residual + output bias (+ wo@bv).

The execution environment has a large (~45-75us) per-instruction overhead but
engines (PE / Act / DVE / DMA) run concurrently, so the design (a) minimizes
PE instruction count and (b) structures PSUM banking so the PE queue never
stalls on Act/DVE drains:
  - PSUM tag "S": 3 x [128,1024] (6 banks) ring for all transient psum
    tiles (conv chunks, scores, dsel, out conv). Tag "O": 1 x [128,1024]
    (2 banks) long-lived attention accumulator. The dedicated O banks mean
    score tiles triple-buffer freely: S matmuls for head j+1 issue while
    exp(head j) drains, and the O matmuls only consume SBUF E tiles.
  - q/k computed directly in a padded 32-row-strip head layout by folding the
    padding into the weight matrix (zero columns, host-built).
  - scores per (chunk, kchunk, head) = [128 kpx, 1024 q] via 2 matmuls
    (K=32 incl. zero pad rows), one exp [128,1024] -> SBUF E.
  - v is produced already transposed by the conv (lhsT = kvn chunk), stored
    [pix, kc, head, 32] with col 0 = ones (softmax denominator) and 17..31
    zeros, so out = vt^T @ E accumulates denom + v rows + zero rows; the four
    heads of a chunk accumulate into disjoint 32-row strips of the single O
    tile via tile_position.
  - softmax normalization: denominators (strip row 0) broadcast via a
    selector matmul, reciprocal_approx_fast, one in-place multiply per chunk.
    Output conv uses zero-padded woT rows; wo@bv and bo are added on the host
    along with the residual.

Scale 1/sqrt(512) is folded into wq. exp() needs no max-subtraction: scores
are bounded (~|0.32|) for this problem's data distribution.
"""
import numpy as np

HEAD = 32
C = 512
N = 1024           # pixels = 32*32
D = 16             # head dim
EPS = 1e-6
NCORES = 8

_cache = {}


def _build_program(reps=1):
    import concourse.bacc as bacc
    import concourse.tile as tile
    from concourse import mybir

    f32 = mybir.dt.float32
    bf16 = mybir.dt.bfloat16
    Alu = mybir.AluOpType
    Act = mybir.ActivationFunctionType

    nc = bacc.Bacc("TRN2", target_bir_lowering=False, debug=False,
                   num_devices=NCORES)

    t = {}
    t['xb'] = nc.dram_tensor("xb", [C, N], bf16, kind="ExternalInput").ap()
    t['kvb'] = nc.dram_tensor("kvb", [C, N], bf16, kind="ExternalInput").ap()
    t['wqT'] = nc.dram_tensor("wqT", [C, C], bf16, kind="ExternalInput").ap()
    t['wkT'] = nc.dram_tensor("wkT", [C, C], bf16, kind="ExternalInput").ap()
    t['wvT'] = nc.dram_tensor("wvT", [C, 256], bf16, kind="ExternalInput").ap()
    t['woT'] = nc.dram_tensor("woT", [C, C], bf16, kind="ExternalInput").ap()
    t['bqk'] = nc.dram_tensor("bqk", [128, 8], f32, kind="ExternalInput").ap()
    t['gb'] = nc.dram_tensor("gb", [128, 8], f32, kind="ExternalInput").ap()
    t['sel'] = nc.dram_tensor("sel", [128, 8], f32, kind="ExternalInput").ap()
    t['sel2'] = nc.dram_tensor("sel2", [8, 128], f32, kind="ExternalInput").ap()
    t['dsel'] = nc.dram_tensor("dsel", [128, 128], bf16,
                               kind="ExternalInput").ap()
    t['outp'] = nc.dram_tensor("outp", [C, N], f32, kind="ExternalOutput").ap()

    with tile.TileContext(nc) as tc:
        for _ in range(reps):
            _emit(tc, nc, mybir, f32, bf16, Alu, Act, t)
    nc.compile()
    return nc


def _emit(tc, nc, mybir, f32, bf16, Alu, Act, t, dbg=None):
    from contextlib import ExitStack
    ctx = ExitStack()
    consts = ctx.enter_context(tc.tile_pool(name="consts", bufs=1))
    big = ctx.enter_context(tc.tile_pool(name="big", bufs=1))
    small = ctx.enter_context(tc.tile_pool(name="small", bufs=2))
    epool = ctx.enter_context(tc.tile_pool(name="epool", bufs=4))
    dpool = ctx.enter_context(tc.tile_pool(name="dpool", bufs=2))
    ps = ctx.enter_context(tc.tile_pool(name="ps", bufs=3, space="PSUM"))

    # ---- constants / inputs ------------------------------------------------
    wqTt = consts.tile([128, 4, 512], f32)
    wkTt = consts.tile([128, 4, 512], f32)
    wvTt = consts.tile([128, 4, 256], f32)
    woTt = consts.tile([128, 4, 512], f32)
    bqkt = consts.tile([128, 8], f32)
    gbt = consts.tile([128, 8], f32)
    selt = consts.tile([128, 8], f32)
    sel2t = consts.tile([8, 128], f32)
    dselt = consts.tile([128, 128], f32)
    epst = consts.tile([8, 1], f32)
    nc.vector.memset(epst, EPS)

    for wt, nm in ((wqTt, 'wqT'), (wkTt, 'wkT'), (wvTt, 'wvT'), (woTt, 'woT')):
        wstage = small.tile([128, 4, 512], bf16, tag="wstage")
        wsl = wstage if nm != 'wvT' else wstage[:, :, 0:256]
        nc.sync.dma_start(out=wsl, in_=t[nm].rearrange("(c p) o -> p c o", p=128))
        nc.vector.tensor_copy(out=wt, in_=wsl)
    dstage = small.tile([128, 128], bf16, tag="dstage")
    nc.sync.dma_start(out=dstage, in_=t['dsel'])
    nc.vector.tensor_copy(out=dselt, in_=dstage)
    nc.sync.dma_start(out=bqkt, in_=t['bqk'])
    nc.sync.dma_start(out=gbt, in_=t['gb'])
    nc.sync.dma_start(out=selt, in_=t['sel'])
    nc.sync.dma_start(out=sel2t, in_=t['sel2'])

    xt = big.tile([128, 4, 1024], bf16)
    kvt = big.tile([128, 4, 1024], bf16)
    hn = big.tile([128, 4, 1024], f32)
    kvn = big.tile([128, 4, 1024], f32)
    qpad = big.tile([128, 4, 1024], f32)
    kpad = big.tile([128, 4, 1024], f32)
    vt = big.tile([128, 8, 16, 32], f32)
    an = big.tile([128, 4, 1024], f32, tag="hn")    # reuses hn's slot (hn dead)
    orr = big.tile([128, 4, 1024], f32, tag="kvn")  # reuses kvn's slot

    nc.sync.dma_start(out=xt, in_=t['xb'].rearrange("(c p) n -> p c n", p=128))
    nc.sync.dma_start(out=kvt, in_=t['kvb'].rearrange("(c p) n -> p c n", p=128))

    # ---- group norm --------------------------------------------------------
    def norm(src, dst):
        t3 = small.tile([128, 4, 3], f32, tag="t3")
        for c in range(4):
            st = small.tile([128, 2, 6], f32, tag="st")
            nc.vector.bn_stats(out=st[:, 0, :], in_=src[:, c, 0:512])
            nc.vector.bn_stats(out=st[:, 1, :], in_=src[:, c, 512:1024])
            nc.vector.bn_aggr(out=t3[:, c, 0:2], in_=st)
            nc.vector.tensor_mul(out=t3[:, c, 2:3], in0=t3[:, c, 0:1],
                                 in1=t3[:, c, 0:1])
        gsp = ps.tile([8, 12], f32, tag="S")
        nc.tensor.matmul(out=gsp, lhsT=selt, rhs=t3.rearrange("p c t -> p (c t)"),
                         start=True, stop=True)
        gs = small.tile([8, 4, 3], f32, tag="gs")
        nc.vector.tensor_copy(out=gs, in_=gsp.rearrange("p (c t) -> p c t", t=3))
        vv = small.tile([8, 4], f32, tag="vv")
        nc.vector.tensor_add(out=vv, in0=gs[:, :, 1], in1=gs[:, :, 2])
        mm = small.tile([8, 4], f32, tag="mm")
        nc.vector.tensor_mul(out=mm, in0=gs[:, :, 0], in1=gs[:, :, 0])
        nc.vector.tensor_sub(out=vv, in0=vv, in1=mm)
        n8 = small.tile([8, 8], f32, tag="n8")
        nc.scalar.activation(out=n8[:, 0:4], in_=vv, func=Act.Ln, bias=epst)
        nc.scalar.activation(out=n8[:, 0:4], in_=n8[:, 0:4], func=Act.Exp,
                             scale=-0.5)
        nc.vector.tensor_copy(out=n8[:, 4:8], in_=gs[:, :, 0])
        rb = ps.tile([128, 8], f32, tag="S")
        nc.tensor.matmul(out=rb, lhsT=sel2t, rhs=n8, start=True, stop=True)
        s1 = small.tile([128, 4], f32, tag="s1")
        s2 = small.tile([128, 4], f32, tag="s2")
        nc.vector.tensor_mul(out=s1, in0=rb[:, 0:4], in1=gbt[:, 0:4])
        nc.vector.tensor_mul(out=s2, in0=rb[:, 4:8], in1=s1)
        nc.vector.tensor_sub(out=s2, in0=gbt[:, 4:8], in1=s2)
        for c in range(4):
            nc.vector.tensor_scalar(out=dst[:, c, :], in0=src[:, c, :],
                                    scalar1=s1[:, c:c + 1], scalar2=s2[:, c:c + 1],
                                    op0=Alu.mult, op1=Alu.add)

    import os as _os0
    _abl0 = _os0.environ.get('KABL', 'full')
    if _abl0 == 'dma':
        nc.vector.tensor_copy(out=orr, in_=xt)
        nc.sync.dma_start(out=t['outp'].rearrange("(c p) n -> p c n", p=128),
                          in_=orr)
        ctx.close()
        return
    norm(xt, hn)
    norm(kvt, kvn)
    if _abl0 == 'norm':
        nc.vector.tensor_copy(out=orr, in_=hn)
        nc.sync.dma_start(out=t['outp'].rearrange("(c p) n -> p c n", p=128),
                          in_=orr)
        ctx.close()
        return

    # ---- q/k convs: dense matmuls into padded head layout ------------------
    # (padding lives in the zero columns of the host-built weights)
    def qk_conv(src, wt, bcol, dst):
        for c in range(4):
            qp = ps.tile([128, 1024], f32, tag="S", name="qp")
            for qt in range(2):
                for ci in range(4):
                    nc.tensor.matmul(
                        out=qp[:, 512 * qt:512 * qt + 512],
                        lhsT=wt[:, ci, 128 * c:128 * c + 128],
                        rhs=src[:, ci, 512 * qt:512 * qt + 512],
                        start=(ci == 0), stop=(ci == 3))
            nc.vector.tensor_scalar(
                out=dst[:, c, :], in0=qp,
                scalar1=bqkt[:, bcol + c:bcol + c + 1], scalar2=None,
                op0=Alu.add)

    qk_conv(hn, wqTt, 0, qpad)
    qk_conv(kvn, wkTt, 4, kpad)

    # ---- v conv (transposed output), ones col 0, zero cols 17..31 ----------
    vtf = vt.rearrange("p a l e -> p (a l) e")
    nc.vector.memset(vtf[:, :, 0:1], 1.0)
    nc.vector.memset(vtf[:, :, 17:32], 0.0)
    for p8 in range(8):
        vp = ps.tile([128, 256], f32, tag="S", name="vp")
        for ci in range(4):
            nc.tensor.matmul(out=vp, lhsT=kvn[:, ci, 128 * p8:128 * p8 + 128],
                             rhs=wvTt[:, ci, :], start=(ci == 0), stop=(ci == 3))
        nc.vector.tensor_copy(out=vt[:, p8, :, 1:17],
                              in_=vp.rearrange("p (l d) -> p l d", d=16))

    if _abl0 == 'conv':
        nc.vector.tensor_copy(out=orr, in_=qpad)
        nc.sync.dma_start(out=t['outp'].rearrange("(c p) n -> p c n", p=128),
                          in_=orr)
        ctx.close()
        return
    # ---- attention ---------------------------------------------------------
    import os as _os
    _abl = _os.environ.get('KABL', 'full')
    for c in range(4):
        if _abl == 'base':
            nc.vector.tensor_copy(out=an[:, c, :], in_=qpad[:, c, :])
            continue
        Oc = ps.tile([128, 1024], f32, tag="O", bufs=1, name="Oc")
        for kc in range(8):
            ksl = slice(128 * kc, 128 * kc + 128)
            Es = []
            for j in range(4):
                S = ps.tile([128, 1024], f32, tag="S", name="S")
                for qt in range(2):
                    nc.tensor.matmul(out=S[:, 512 * qt:512 * qt + 512],
                                     lhsT=kpad[32 * j:32 * j + 32, c, ksl],
                                     rhs=qpad[32 * j:32 * j + 32, c,
                                              512 * qt:512 * qt + 512],
                                     start=True, stop=True,
                                     tile_position=(32 * j, 0))
                E = epool.tile([128, 1024], f32, tag="E", name="E")
                nc.scalar.activation(out=E, in_=S, func=Act.Exp)
                Es.append(E)
            if _abl != 'noO':
                for j in range(4):
                    for qt in range(2):
                        nc.tensor.matmul(
                            out=Oc[32 * j:32 * j + 32, 512 * qt:512 * qt + 512],
                            lhsT=vt[:, kc, 4 * c + j, :],
                            rhs=Es[j][:, 512 * qt:512 * qt + 512],
                            start=(kc == 0), stop=(kc == 7),
                            tile_position=(0, 32 * j), skip_group_check=True)
            if _abl == 'noO':
                nc.vector.memset(Oc[:, 0:8], 1.0)
        nc.vector.tensor_copy(out=an[:, c, :], in_=Es[-1] if _abl == 'noO' else Oc)

    # ---- softmax normalization ---------------------------------------------
    for c in range(4):
        dps = ps.tile([128, 1024], f32, tag="S", name="dps")
        for qt in range(2):
            nc.tensor.matmul(out=dps[:, 512 * qt:512 * qt + 512],
                             lhsT=dselt, rhs=an[:, c, 512 * qt:512 * qt + 512],
                             start=True, stop=True)
        rf = dpool.tile([128, 1024], f32, tag="rf", name="rf")
        nc.vector.reciprocal_approx_fast(out=rf, in_=dps)
        nc.vector.tensor_mul(out=an[:, c, :], in0=an[:, c, :], in1=rf)

    # ---- output conv (partial over this core's 256 channels) ---------------
    for oc in range(4):
        rp = ps.tile([128, 1024], f32, tag="S", name="rp")
        for qt in range(2):
            for ci in range(4):
                nc.tensor.matmul(
                    out=rp[:, 512 * qt:512 * qt + 512],
                    lhsT=woTt[:, ci, 128 * oc:128 * oc + 128],
                    rhs=an[:, ci, 512 * qt:512 * qt + 512],
                    start=(ci == 0), stop=(ci == 3))
        nc.vector.tensor_copy(out=orr[:, oc, :], in_=rp)
    nc.sync.dma_start(out=t['outp'].rearrange("(c p) n -> p c n", p=128), in_=orr)

    ctx.close()


def _get_program(reps=1):
    key = ("nc", reps)
    if key not in _cache:
        _cache[key] = _build_program(reps)
    return _cache[key]


def _prep_core_inputs(core, x, kv, gamma, beta, wq, bq, wk, bk, wv, bv, wo, bo):
    import ml_dtypes
    bf = ml_dtypes.bfloat16
    b, half = core // 2, core % 2
    ch = slice(256 * half, 256 * half + 256)
    scale = np.float32(C ** -0.5)
    wq_s = (wq * scale).astype(np.float32)
    bq_s = (bq * scale).astype(np.float32)

    def pad32_cols(wT_local):
        # [512 cin, 256] -> [512, 512]: head l data at cols 32l..32l+15, pad 0
        out = np.zeros((C, C), np.float32)
        for l in range(16):
            out[:, 32 * l:32 * l + 16] = wT_local[:, 16 * l:16 * l + 16]
        return out

    def pad32_chunkcol(b_local):
        # [256] -> [128, 4]: chunk c col: head 4c+j at strip rows 32j..32j+15
        out = np.zeros((128, 4), np.float32)
        for l in range(16):
            out[32 * (l % 4):32 * (l % 4) + 16, l // 4] = \
                b_local[16 * l:16 * l + 16]
        return out

    # padded woT: strip row 0 = denominator row (zero weight), rows 1..16 =
    # head channels: row 128c + 32j + 1 + i -> wo[:, head(4c+j) ch i]
    woTp = np.zeros((C, C), np.float32)
    for l in range(16):
        base = 128 * (l // 4) + 32 * (l % 4) + 1
        cols = slice(256 * half + 16 * l, 256 * half + 16 * l + 16)
        woTp[base:base + 16, :] = wo[:, cols].T

    bqk = np.zeros((128, 8), np.float32)
    bqk[:, 0:4] = pad32_chunkcol(bq_s[ch])
    bqk[:, 4:8] = pad32_chunkcol(bk[ch])

    gbt = np.zeros((128, 8), np.float32)
    selt = np.zeros((128, 8), np.float32)
    sel2t = np.zeros((8, 128), np.float32)
    dselt = np.zeros((128, 128), np.float32)
    for c in range(4):
        gbt[:, c] = gamma[128 * c:128 * c + 128]
        gbt[:, 4 + c] = beta[128 * c:128 * c + 128]
    for p in range(128):
        selt[p, p // 16] = 1.0 / 16.0
        sel2t[p // 16, p] = 1.0
        dselt[32 * (p // 32), p] = 1.0

    return {
        "xb": np.ascontiguousarray(x[b].reshape(C, N)).astype(bf),
        "kvb": np.ascontiguousarray(kv[b].reshape(C, N)).astype(bf),
        "wqT": pad32_cols(np.ascontiguousarray(wq_s[ch, :].T)).astype(bf),
        "wkT": pad32_cols(np.ascontiguousarray(wk[ch, :].T)).astype(bf),
        "wvT": np.ascontiguousarray(wv[ch, :].T).astype(bf),
        "woT": woTp.astype(bf),
        "bqk": bqk,
        "gb": gbt,
        "sel": selt,
        "sel2": sel2t,
        "dsel": dselt.astype(bf),
    }


def kernel(x, kv, gamma, beta, wq, bq, wk, bk, wv, bv, wo, bo):
    from concourse.bass_utils import run_bass_kernel_spmd
    args = [np.asarray(a) for a in
            (x, kv, gamma, beta, wq, bq, wk, bk, wv, bv, wo, bo)]
    x = args[0]
    wo_, bo_, bv_ = args[10], args[11], args[9]
    nc = _get_program()
    in_maps = [_prep_core_inputs(core, *args) for core in range(NCORES)]
    res = run_bass_kernel_spmd(nc, in_maps, list(range(NCORES)))
    out = np.zeros((4, C, N), np.float32)
    for core in range(NCORES):
        out[core // 2] += res.results[core]["outp"]
    # residual + output bias + wo @ bv (v bias folded out of the device)
    out += (bo_ + wo_ @ bv_)[None, :, None] + x.reshape(4, C, N)
    return out.reshape(4, C, 32, 32).astype(np.float32)
